# revision 21
# baseline (speedup 1.0000x reference)
"""Trainium2 Bass kernel for the PerforantHebb AHA module.

Math (reference.py):
    pre     = ec @ W^T                      (B, CA3)
    targets = dg + pre                      -> output pre_pc_cue
    outer   = targets^T @ ec / B            (CA3, EC)
    m       = mean(targets, 0)              (CA3,)
    delta   = ec @ outer^T - pre * m[None]  (B, CA3)   [dW folded algebraically]
    ec_ca3_loss = pc_cue_loss = LR^2 * mean(delta^2);  dg_ca3_loss = 0

Sharding: CA3 split 8 ways (256 rows/core). Everything is local per core —
no collectives. Per core, three matmul phases, all with the batch dim on
PSUM partitions (no on-chip transposes):
  M1: pre[b]    = sum_k ecT[k,b]^T @ W_sT[k]      (K=EC)
  M2: outerT[e] = sum_b ecN[b,e]^T @ targets[b]   (K=B)
  M3: delta[b]  = sum_k ecT[k,b]^T @ outerT[k]    (K=EC)
Host feeds ec in both layouts (natural + transposed, bf16) plus per-core
W_sT / dg slices. M1 is k-outer with 8 concurrent psum groups so the PE
chases the ecT DMA stream; M2 runs in four e-quarters of 8 psum groups so
its first quarter chases the ecN stream.
"""

import numpy as np
import ml_dtypes

import concourse.bass as bass
import concourse.bacc as bacc
import concourse.mybir as mybir
import concourse.tile as tile
from concourse.bass_utils import run_bass_kernel_spmd

B, EC, CA3 = 1024, 4096, 2048
LR = 0.01
N_CORES = 8
S = CA3 // N_CORES          # 256 ca3 rows per core
P = 128
KT = EC // P                # 32 k-tiles over EC
NB = B // P                 # 8 b-chunks of 128
KP = KT // 2                # k-pair DMA granules

F32 = mybir.dt.float32
MMDT = mybir.dt.bfloat16    # matmul operand dtype
MMNP = ml_dtypes.bfloat16

_CACHE = {}


def _build(phases=4):
    import os
    phases = int(os.environ.get("KERNEL_PHASES", phases))
    nc = bacc.Bacc("TRN2", target_bir_lowering=False, debug=False,
                   num_devices=N_CORES)
    ecT = nc.dram_tensor("ecT", (EC, B), MMDT, kind="ExternalInput").ap()
    ecN = nc.dram_tensor("ecN", (B, EC), MMDT, kind="ExternalInput").ap()
    WT = nc.dram_tensor("WT", (EC, S), MMDT, kind="ExternalInput").ap()
    dgN = nc.dram_tensor("dgN", (B, S), F32, kind="ExternalInput").ap()
    outN = nc.dram_tensor("outN", (B, S), F32, kind="ExternalOutput").ap()
    loss = nc.dram_tensor("loss", (P, 1), F32, kind="ExternalOutput").ap()

    with tile.TileContext(nc) as tc:
        with (
            tc.tile_pool(name="res", bufs=1) as res,
            tc.tile_pool(name="tmp", bufs=4) as tmp,
            tc.tile_pool(name="ps", bufs=8, space="PSUM") as ps,
        ):
            # ---- resident tiles ----
            sb_ecT = [res.tile([P, 2, B], MMDT, tag=f"ecT{kp}", name=f"ecT{kp}")
                      for kp in range(KP)]
            sb_WT = [res.tile([P, 2, S], MMDT, tag=f"WT{kp}", name=f"WT{kp}")
                     for kp in range(KP)]
            sb_ecN = [res.tile([P, EC], MMDT, tag=f"ecN{b}", name=f"ecN{b}")
                      for b in range(NB)]
            sb_dg = [res.tile([P, S], F32, tag=f"dg{b}", name=f"dg{b}")
                     for b in range(NB)]

            # dg early on the gpsimd (SWDGE) queue so M1 epilogues never wait.
            for b in range(NB):
                nc.gpsimd.dma_start(sb_dg[b][:], dgN[b * P:(b + 1) * P, :])
            # Input stream striped across TWO HWDGE queues (sync + scalar):
            # ecT/WT pairs first (they pace M1), then ecN chunks (they pace
            # M2's first quarter). Output writes go on the gpsimd queue.
            qs = [nc.sync, nc.scalar]
            for kp in range(KP):
                q = qs[kp % 2]
                q.dma_start(
                    sb_ecT[kp][:],
                    ecT[kp * 2 * P:(kp + 1) * 2 * P, :].rearrange(
                        "(two p) b -> p two b", p=P))
                q.dma_start(
                    sb_WT[kp][:],
                    WT[kp * 2 * P:(kp + 1) * 2 * P, :].rearrange(
                        "(two p) s -> p two s", p=P))
            for b in range(NB):
                qs[b % 2].dma_start(sb_ecN[b][:], ecN[b * P:(b + 1) * P, :])

            sb_tg = [res.tile([P, S], F32, tag=f"tg{b}", name=f"tg{b}")
                     for b in range(NB)]
            sb_tb = [res.tile([P, S], MMDT, tag=f"tb{b}", name=f"tb{b}")
                     for b in range(NB)]
            sb_oT = [res.tile([P, S], MMDT, tag=f"oT{k}", name=f"oT{k}")
                     for k in range(KT)]
            sb_ones = res.tile([P, 1], MMDT, tag="ones", name="ones")
            sb_mrow = res.tile([1, S], F32, tag="mrow", name="mrow")
            sb_mbc = res.tile([P, S], F32, tag="mbc", name="mbc")
            nc.vector.memset(sb_ones[:], 1.0)

            def ecT_sl(k, b):
                return sb_ecT[k // 2][:, k % 2, b * P:(b + 1) * P]

            # ---- M1: k-outer, 8 psum groups chase the ecT/WT stream ----
            p1 = [ps.tile([P, 512], F32, tag="mm", name="mmps")
                  for _ in range(NB)]
            for k in range(KT):
                for b in range(NB):
                    nc.tensor.matmul(
                        p1[b][:, :S], ecT_sl(k, b), sb_WT[k // 2][:, k % 2, :],
                        start=(k == 0), stop=(k == KT - 1))
            for b in range(NB):
                # targets = pre + dg (f32, main output) + bf16 copy for M2
                nc.vector.tensor_add(sb_tg[b][:], p1[b][:, :S], sb_dg[b][:])
                nc.vector.tensor_copy(sb_tb[b][:], sb_tg[b][:])
                nc.gpsimd.dma_start(outN[b * P:(b + 1) * P, :], sb_tg[b][:])

            # ---- M2: outerT[e] = sum_b ecN[b,e]^T @ targets[b], scaled 1/B.
            # Four e-quarters of 8 concurrent psum groups (one per bank —
            # PSUM allows one accumulation group per 2KiB zero region). ----
            EH = 8
            for h in range(4 if phases >= 3 else 0):
                p2 = [ps.tile([P, 512], F32, tag="mm", name="mmps")
                      for _ in range(EH)]
                for b in range(NB):
                    for ei in range(EH):
                        e = h * EH + ei
                        nc.tensor.matmul(
                            p2[ei][:, :S], sb_ecN[b][:, e * P:(e + 1) * P],
                            sb_tb[b][:],
                            start=(b == 0), stop=(b == NB - 1))
                for ei in range(EH):
                    e = h * EH + ei
                    nc.vector.tensor_scalar_mul(
                        sb_oT[e][:], p2[ei][:, :S], 1.0 / B)
                if h == 0 and phases >= 2:
                    # m = mean(targets, 0) broadcast to all partitions, then
                    # pm[b] = (targets[b] - dg[b]) * m / B, overwriting dg.
                    # Emitted inside M2 so the tiny PE matmuls slot between
                    # quarters; pm is only needed by the M3 epilogues.
                    pmps = ps.tile([P, 512], F32, tag="mm", name="mmps")
                    for b in range(NB):
                        nc.tensor.matmul(
                            pmps[:1, :S], sb_ones[:], sb_tb[b][:],
                            start=(b == 0), stop=(b == NB - 1))
                    nc.vector.tensor_scalar_mul(
                        sb_mrow[:], pmps[:1, :S], 1.0 / B)
                    nc.gpsimd.partition_broadcast(sb_mbc[:], sb_mrow[:])
                    for b in range(NB):
                        nc.vector.tensor_sub(sb_dg[b][:], sb_tg[b][:],
                                             sb_dg[b][:])
                        nc.vector.tensor_mul(sb_dg[b][:], sb_dg[b][:],
                                             sb_mbc[:])

            # ---- M3: delta[b] = sum_k ecT[k,b]^T @ outerT[k] - pm[b], in
            # two waves of 4 groups so wave A epilogues overlap wave B ----
            part_prev = None
            for w in range(2 if phases >= 4 else 0):
                p3 = [ps.tile([P, 512], F32, tag="mm", name="mmps")
                      for _ in range(4)]
                for k in range(KT):
                    for g in range(4):
                        nc.tensor.matmul(
                            p3[g][:, :S], ecT_sl(k, w * 4 + g), sb_oT[k][:],
                            start=(k == 0), stop=(k == KT - 1))
                for g in range(4):
                    b = w * 4 + g
                    d = tmp.tile([P, 512], F32, tag="d", name="d")
                    nc.vector.tensor_sub(d[:, :S], p3[g][:, :S], sb_dg[b][:])
                    dsq = tmp.tile([P, 512], F32, tag="dsq", name="dsq")
                    nc.vector.tensor_mul(dsq[:, :S], d[:, :S], d[:, :S])
                    part = tmp.tile([P, 1], F32, tag=f"part{b}",
                                    name=f"part{b}")
                    nc.vector.tensor_reduce(
                        part[:], dsq[:, :S], axis=mybir.AxisListType.X,
                        op=mybir.AluOpType.add)
                    if part_prev is not None:
                        nc.vector.tensor_add(part[:], part[:], part_prev[:])
                    part_prev = part

            # ---- write per-partition loss partials; host sums them ----
            if part_prev is not None:
                nc.gpsimd.dma_start(loss[:, :], part_prev[:])
            else:
                zl = tmp.tile([P, 1], F32, tag="zl", name="zl")
                nc.vector.memset(zl[:], 0.0)
                nc.gpsimd.dma_start(loss[:, :], zl[:])

    nc.compile()
    return nc


def _get_nc():
    if "nc" not in _CACHE:
        _CACHE["nc"] = _build()
    return _CACHE["nc"]


def _in_maps(ec, dg, W):
    ecN_h = ec.astype(MMNP)
    ecT_h = np.ascontiguousarray(ec.T).astype(MMNP)
    maps = []
    for i in range(N_CORES):
        Ws = W[i * S:(i + 1) * S, :]
        maps.append({
            "ecT": ecT_h,
            "ecN": ecN_h,
            "WT": np.ascontiguousarray(Ws.T).astype(MMNP),
            "dgN": np.ascontiguousarray(dg[:, i * S:(i + 1) * S]),
        })
    return maps


def kernel(ec_inputs: np.ndarray, dg_inputs: np.ndarray, W_ec: np.ndarray):
    nc = _get_nc()
    ec = np.asarray(ec_inputs, dtype=np.float32)
    dg = np.asarray(dg_inputs, dtype=np.float32)
    W = np.asarray(W_ec, dtype=np.float32)

    res = run_bass_kernel_spmd(nc, _in_maps(ec, dg, W), list(range(N_CORES)))

    pre_pc_cue = np.empty((B, CA3), dtype=np.float32)
    loss_sum = 0.0
    for i in range(N_CORES):
        pre_pc_cue[:, i * S:(i + 1) * S] = res.results[i]["outN"]
        loss_sum += float(res.results[i]["loss"].sum())

    loss = np.float32(LR * LR * loss_sum / (B * CA3))
    return (pre_pc_cue, np.float32(0.0), loss, loss)


# revision 23
# speedup vs baseline: 1.0949x; 1.0949x over previous
"""Trainium2 Bass kernel for the PerforantHebb AHA module.

Math (reference.py):
    pre     = ec @ W^T                      (B, CA3)
    targets = dg + pre                      -> output pre_pc_cue
    outer   = targets^T @ ec / B            (CA3, EC)
    m       = mean(targets, 0)              (CA3,)
    delta   = ec @ outer^T - pre * m[None]  (B, CA3)   [dW folded algebraically]
    ec_ca3_loss = pc_cue_loss = LR^2 * mean(delta^2);  dg_ca3_loss = 0

Sharding: CA3 split 8 ways (256 rows/core). Everything is local per core —
no collectives. Per core, three matmul phases, all with the batch dim on
PSUM partitions (no on-chip transposes):
  M1: pre[b]    = sum_k ecT[k,b]^T @ W_sT[k]      (K=EC)
  M2: outerT[e] = sum_b ecN[b,e]^T @ targets[b]   (K=B)
  M3: delta[b]  = sum_k ecT[k,b]^T @ outerT[k]    (K=EC)
Host feeds ec in both layouts (natural + transposed, bf16) plus per-core
W_sT / dg slices. M1 is k-outer with 8 concurrent psum groups so the PE
chases the ecT DMA stream; M2 runs in four e-quarters of 8 psum groups so
its first quarter chases the ecN stream.
"""

import numpy as np
import ml_dtypes

import concourse.bass as bass
import concourse.bacc as bacc
import concourse.mybir as mybir
import concourse.tile as tile
from concourse.bass_utils import run_bass_kernel_spmd

B, EC, CA3 = 1024, 4096, 2048
LR = 0.01
N_CORES = 8
S = CA3 // N_CORES          # 256 ca3 rows per core
P = 128
KT = EC // P                # 32 k-tiles over EC
NB = B // P                 # 8 b-chunks of 128
KP = KT // 2                # k-pair DMA granules

F32 = mybir.dt.float32
MMDT = mybir.dt.bfloat16    # matmul operand dtype
MMNP = ml_dtypes.bfloat16

_CACHE = {}


def _build(phases=4):
    import os
    phases = int(os.environ.get("KERNEL_PHASES", phases))
    nc = bacc.Bacc("TRN2", target_bir_lowering=False, debug=False,
                   num_devices=N_CORES)
    ecT = nc.dram_tensor("ecT", (EC, B), MMDT, kind="ExternalInput").ap()
    ecN = nc.dram_tensor("ecN", (B, EC), MMDT, kind="ExternalInput").ap()
    WT = nc.dram_tensor("WT", (EC, S), MMDT, kind="ExternalInput").ap()
    dgN = nc.dram_tensor("dgN", (B, S), F32, kind="ExternalInput").ap()
    outN = nc.dram_tensor("outN", (B, S), F32, kind="ExternalOutput").ap()
    loss = nc.dram_tensor("loss", (P, 1), F32, kind="ExternalOutput").ap()

    with tile.TileContext(nc) as tc:
        with (
            tc.tile_pool(name="res", bufs=1) as res,
            tc.tile_pool(name="tmp", bufs=4) as tmp,
            tc.tile_pool(name="ps", bufs=8, space="PSUM") as ps,
        ):
            # ---- resident tiles ----
            sb_ecT = [res.tile([P, 2, B], MMDT, tag=f"ecT{kp}", name=f"ecT{kp}")
                      for kp in range(KP)]
            sb_WT = [res.tile([P, 2, S], MMDT, tag=f"WT{kp}", name=f"WT{kp}")
                     for kp in range(KP)]
            sb_ecN = [res.tile([P, EC], MMDT, tag=f"ecN{b}", name=f"ecN{b}")
                      for b in range(NB)]
            sb_dg = [res.tile([P, S], F32, tag=f"dg{b}", name=f"dg{b}")
                     for b in range(NB)]

            # dg early on the gpsimd (SWDGE) queue so M1 epilogues never wait.
            for b in range(NB):
                nc.gpsimd.dma_start(sb_dg[b][:], dgN[b * P:(b + 1) * P, :])
            # Input stream striped across TWO HWDGE queues (sync + scalar):
            # ecT/WT pairs first (they pace M1), then ecN chunks (they pace
            # M2's first quarter). Output writes go on the gpsimd queue.
            qs = [nc.sync, nc.scalar]
            for kp in range(KP):
                q = qs[kp % 2]
                q.dma_start(
                    sb_ecT[kp][:],
                    ecT[kp * 2 * P:(kp + 1) * 2 * P, :].rearrange(
                        "(two p) b -> p two b", p=P))
                q.dma_start(
                    sb_WT[kp][:],
                    WT[kp * 2 * P:(kp + 1) * 2 * P, :].rearrange(
                        "(two p) s -> p two s", p=P))
            for b in range(NB):
                qs[b % 2].dma_start(sb_ecN[b][:], ecN[b * P:(b + 1) * P, :])

            sb_tg = [res.tile([P, S], F32, tag=f"tg{b}", name=f"tg{b}")
                     for b in range(NB)]
            sb_tb = [res.tile([P, S], MMDT, tag=f"tb{b}", name=f"tb{b}")
                     for b in range(NB)]
            sb_oT = [res.tile([P, S], MMDT, tag=f"oT{k}", name=f"oT{k}")
                     for k in range(KT)]
            sb_ones = res.tile([P, 1], MMDT, tag="ones", name="ones")
            sb_mrow = res.tile([1, S], F32, tag="mrow", name="mrow")
            sb_mbc = res.tile([P, S], F32, tag="mbc", name="mbc")
            nc.vector.memset(sb_ones[:], 1.0)

            def ecT_sl(k, b):
                return sb_ecT[k // 2][:, k % 2, b * P:(b + 1) * P]

            # ---- M1: k-outer, 8 psum groups chase the ecT/WT stream ----
            p1 = [ps.tile([P, 512], F32, tag="mm", name="mmps")
                  for _ in range(NB)]
            for k in range(KT):
                for b in range(NB):
                    nc.tensor.matmul(
                        p1[b][:, :S], ecT_sl(k, b), sb_WT[k // 2][:, k % 2, :],
                        start=(k == 0), stop=(k == KT - 1))
            for b in range(NB):
                # targets = pre + dg (f32, main output) + bf16 copy for M2
                nc.vector.tensor_add(sb_tg[b][:], p1[b][:, :S], sb_dg[b][:])
                nc.vector.tensor_copy(sb_tb[b][:], sb_tg[b][:])
                nc.gpsimd.dma_start(outN[b * P:(b + 1) * P, :], sb_tg[b][:])

            # ---- M2: outerT[e] = sum_b ecN[b,e]^T @ targets[b], scaled 1/B.
            # Two e-halves of 16 concurrent psum groups: two (128,256)
            # groups share each bank. start=True zeroes the whole 2KiB
            # zero region, so only the bank's first MM sets start and only
            # its last sets stop; the odd group accumulates into the half
            # the even group's start already zeroed. 16 groups in flight
            # lets the first half consume the ecN stream at arrival rate.
            EH = 16
            for h in range(2 if phases >= 3 else 0):
                p2 = [ps.tile([P, 512], F32, tag="mm", name="mmps")
                      for _ in range(EH // 2)]
                for b in range(NB):
                    for ei in range(EH):
                        e = h * EH + ei
                        bank = p2[ei // 2]
                        col = (ei % 2) * S
                        nc.tensor.matmul(
                            bank[:, col:col + S],
                            sb_ecN[b][:, e * P:(e + 1) * P], sb_tb[b][:],
                            start=(b == 0 and ei % 2 == 0),
                            stop=(b == NB - 1 and ei % 2 == 1))
                for ei in range(EH):
                    e = h * EH + ei
                    nc.vector.tensor_scalar_mul(
                        sb_oT[e][:], p2[ei // 2][:, (ei % 2) * S:(ei % 2) * S + S],
                        1.0 / B)
                if h == 0 and phases >= 2:
                    # m = mean(targets, 0) broadcast to all partitions, then
                    # pm[b] = (targets[b] - dg[b]) * m / B, overwriting dg.
                    # Emitted inside M2 so the tiny PE matmuls slot between
                    # quarters; pm is only needed by the M3 epilogues.
                    pmps = ps.tile([P, 512], F32, tag="mm", name="mmps")
                    for b in range(NB):
                        nc.tensor.matmul(
                            pmps[:1, :S], sb_ones[:], sb_tb[b][:],
                            start=(b == 0), stop=(b == NB - 1))
                    nc.vector.tensor_scalar_mul(
                        sb_mrow[:], pmps[:1, :S], 1.0 / B)
                    nc.gpsimd.partition_broadcast(sb_mbc[:], sb_mrow[:])
                    for b in range(NB):
                        nc.vector.tensor_sub(sb_dg[b][:], sb_tg[b][:],
                                             sb_dg[b][:])
                        nc.vector.tensor_mul(sb_dg[b][:], sb_dg[b][:],
                                             sb_mbc[:])

            # ---- M3: delta[b] = sum_k ecT[k,b]^T @ outerT[k] - pm[b], in
            # four waves of 2 groups so wave epilogues overlap the next
            # wave and only the last wave's two chains are exposed ----
            part_prev = None
            for w in range(4 if phases >= 4 else 0):
                p3 = [ps.tile([P, 512], F32, tag="mm", name="mmps")
                      for _ in range(2)]
                for k in range(KT):
                    for g in range(2):
                        nc.tensor.matmul(
                            p3[g][:, :S], ecT_sl(k, w * 2 + g), sb_oT[k][:],
                            start=(k == 0), stop=(k == KT - 1))
                for g in range(2):
                    b = w * 2 + g
                    d = tmp.tile([P, 512], F32, tag="d", name="d")
                    nc.vector.tensor_sub(d[:, :S], p3[g][:, :S], sb_dg[b][:])
                    dsq = tmp.tile([P, 512], F32, tag="dsq", name="dsq")
                    nc.vector.tensor_mul(dsq[:, :S], d[:, :S], d[:, :S])
                    part = tmp.tile([P, 1], F32, tag=f"part{b}",
                                    name=f"part{b}")
                    nc.vector.tensor_reduce(
                        part[:], dsq[:, :S], axis=mybir.AxisListType.X,
                        op=mybir.AluOpType.add)
                    if part_prev is not None:
                        nc.vector.tensor_add(part[:], part[:], part_prev[:])
                    part_prev = part

            # ---- write per-partition loss partials; host sums them ----
            if part_prev is not None:
                nc.gpsimd.dma_start(loss[:, :], part_prev[:])
            else:
                zl = tmp.tile([P, 1], F32, tag="zl", name="zl")
                nc.vector.memset(zl[:], 0.0)
                nc.gpsimd.dma_start(loss[:, :], zl[:])

    nc.compile()
    return nc


def _get_nc():
    if "nc" not in _CACHE:
        _CACHE["nc"] = _build()
    return _CACHE["nc"]


def _in_maps(ec, dg, W):
    ecN_h = ec.astype(MMNP)
    ecT_h = np.ascontiguousarray(ec.T).astype(MMNP)
    maps = []
    for i in range(N_CORES):
        Ws = W[i * S:(i + 1) * S, :]
        maps.append({
            "ecT": ecT_h,
            "ecN": ecN_h,
            "WT": np.ascontiguousarray(Ws.T).astype(MMNP),
            "dgN": np.ascontiguousarray(dg[:, i * S:(i + 1) * S]),
        })
    return maps


def kernel(ec_inputs: np.ndarray, dg_inputs: np.ndarray, W_ec: np.ndarray):
    nc = _get_nc()
    ec = np.asarray(ec_inputs, dtype=np.float32)
    dg = np.asarray(dg_inputs, dtype=np.float32)
    W = np.asarray(W_ec, dtype=np.float32)

    res = run_bass_kernel_spmd(nc, _in_maps(ec, dg, W), list(range(N_CORES)))

    pre_pc_cue = np.empty((B, CA3), dtype=np.float32)
    loss_sum = 0.0
    for i in range(N_CORES):
        pre_pc_cue[:, i * S:(i + 1) * S] = res.results[i]["outN"]
        loss_sum += float(res.results[i]["loss"].sum())

    loss = np.float32(LR * LR * loss_sum / (B * CA3))
    return (pre_pc_cue, np.float32(0.0), loss, loss)


# revision 26
# speedup vs baseline: 1.1022x; 1.0067x over previous
"""Trainium2 Bass kernel for the PerforantHebb AHA module.

Math (reference.py):
    pre     = ec @ W^T                      (B, CA3)
    targets = dg + pre                      -> output pre_pc_cue
    outer   = targets^T @ ec / B            (CA3, EC)
    m       = mean(targets, 0)              (CA3,)
    delta   = ec @ outer^T - pre * m[None]  (B, CA3)   [dW folded algebraically]
    ec_ca3_loss = pc_cue_loss = LR^2 * mean(delta^2);  dg_ca3_loss = 0

Sharding: CA3 split 8 ways (256 rows/core). Everything is local per core —
no collectives. Per core, three matmul phases, all with the batch dim on
PSUM partitions (no on-chip transposes):
  M1: pre[b]    = sum_k ecT[k,b]^T @ W_sT[k]      (K=EC)
  M2: outerT[e] = sum_b ecN[b,e]^T @ targets[b]   (K=B)
  M3: delta[b]  = sum_k ecT[k,b]^T @ outerT[k]    (K=EC)
Host feeds ec in both layouts (natural + transposed, bf16) plus per-core
W_sT / dg slices. M1 is k-outer with 8 concurrent psum groups so the PE
chases the ecT DMA stream; M2 runs in four e-quarters of 8 psum groups so
its first quarter chases the ecN stream.
"""

import numpy as np
import ml_dtypes

import concourse.bass as bass
import concourse.bacc as bacc
import concourse.mybir as mybir
import concourse.tile as tile
from concourse.bass_utils import run_bass_kernel_spmd

B, EC, CA3 = 1024, 4096, 2048
LR = 0.01
N_CORES = 8
S = CA3 // N_CORES          # 256 ca3 rows per core
P = 128
KT = EC // P                # 32 k-tiles over EC
NB = B // P                 # 8 b-chunks of 128
KP = KT // 2                # k-pair DMA granules

F32 = mybir.dt.float32
MMDT = mybir.dt.bfloat16    # matmul operand dtype
MMNP = ml_dtypes.bfloat16

_CACHE = {}


def _build(phases=4):
    import os
    phases = int(os.environ.get("KERNEL_PHASES", phases))
    nc = bacc.Bacc("TRN2", target_bir_lowering=False, debug=False,
                   num_devices=N_CORES)
    ecT = nc.dram_tensor("ecT", (EC, B), MMDT, kind="ExternalInput").ap()
    ecN = nc.dram_tensor("ecN", (B, EC), MMDT, kind="ExternalInput").ap()
    WT = nc.dram_tensor("WT", (EC, S), MMDT, kind="ExternalInput").ap()
    dgN = nc.dram_tensor("dgN", (B, S), F32, kind="ExternalInput").ap()
    outN = nc.dram_tensor("outN", (B, S), F32, kind="ExternalOutput").ap()
    loss = nc.dram_tensor("loss", (P, 1), F32, kind="ExternalOutput").ap()

    with tile.TileContext(nc) as tc:
        with (
            tc.tile_pool(name="res", bufs=1) as res,
            tc.tile_pool(name="tmp", bufs=4) as tmp,
            tc.tile_pool(name="ps", bufs=8, space="PSUM") as ps,
        ):
            # ---- resident tiles ----
            sb_ecT = [res.tile([P, 2, B], MMDT, tag=f"ecT{kp}", name=f"ecT{kp}")
                      for kp in range(KP)]
            sb_WT = [res.tile([P, 2, S], MMDT, tag=f"WT{kp}", name=f"WT{kp}")
                     for kp in range(KP)]
            sb_ecN = [res.tile([P, EC], MMDT, tag=f"ecN{b}", name=f"ecN{b}")
                      for b in range(NB)]
            sb_dg = [res.tile([P, S], F32, tag=f"dg{b}", name=f"dg{b}")
                     for b in range(NB)]

            # All DMA stays on the two HWDGE queues (sync + scalar): any
            # SWDGE (gpsimd) DMA costs a ~7us gpsimd drain at kernel exit.
            # dg first (small, M1 epilogues need it), then ecT/WT pairs
            # (they pace M1), then ecN chunks (they pace M2's first half).
            qs = [nc.sync, nc.scalar]
            for b in range(NB):
                qs[b % 2].dma_start(sb_dg[b][:], dgN[b * P:(b + 1) * P, :])
            for kp in range(KP):
                q = qs[kp % 2]
                q.dma_start(
                    sb_ecT[kp][:],
                    ecT[kp * 2 * P:(kp + 1) * 2 * P, :].rearrange(
                        "(two p) b -> p two b", p=P))
                q.dma_start(
                    sb_WT[kp][:],
                    WT[kp * 2 * P:(kp + 1) * 2 * P, :].rearrange(
                        "(two p) s -> p two s", p=P))
            for b in range(NB):
                qs[b % 2].dma_start(sb_ecN[b][:], ecN[b * P:(b + 1) * P, :])

            sb_tg = [res.tile([P, S], F32, tag=f"tg{b}", name=f"tg{b}")
                     for b in range(NB)]
            sb_tb = [res.tile([P, S], MMDT, tag=f"tb{b}", name=f"tb{b}")
                     for b in range(NB)]
            sb_oT = [res.tile([P, S], MMDT, tag=f"oT{k}", name=f"oT{k}")
                     for k in range(KT)]
            sb_ones = res.tile([P, 1], MMDT, tag="ones", name="ones")
            sb_mrow = res.tile([1, S], F32, tag="mrow", name="mrow")
            sb_mbc = res.tile([P, S], F32, tag="mbc", name="mbc")
            nc.vector.memset(sb_ones[:], 1.0)

            def ecT_sl(k, b):
                return sb_ecT[k // 2][:, k % 2, b * P:(b + 1) * P]

            # ---- M1: k-outer, 8 psum groups chase the ecT/WT stream ----
            p1 = [ps.tile([P, 512], F32, tag="mm", name="mmps")
                  for _ in range(NB)]
            for k in range(KT):
                for b in range(NB):
                    nc.tensor.matmul(
                        p1[b][:, :S], ecT_sl(k, b), sb_WT[k // 2][:, k % 2, :],
                        start=(k == 0), stop=(k == KT - 1))
            for b in range(NB):
                # targets = pre + dg (f32, main output) + bf16 copy for M2
                nc.vector.tensor_add(sb_tg[b][:], p1[b][:, :S], sb_dg[b][:])
                nc.vector.tensor_copy(sb_tb[b][:], sb_tg[b][:])
                qs[b % 2].dma_start(outN[b * P:(b + 1) * P, :], sb_tg[b][:])

            # ---- M2: outerT[e] = sum_b ecN[b,e]^T @ targets[b], scaled 1/B.
            # Two e-halves of 16 concurrent psum groups: two (128,256)
            # groups share each bank. start=True zeroes the whole 2KiB
            # zero region, so only the bank's first MM sets start and only
            # its last sets stop; the odd group accumulates into the half
            # the even group's start already zeroed. 16 groups in flight
            # lets the first half consume the ecN stream at arrival rate.
            EH = 16
            for h in range(2 if phases >= 3 else 0):
                p2 = [ps.tile([P, 512], F32, tag="mm", name="mmps")
                      for _ in range(EH // 2)]
                for b in range(NB):
                    for ei in range(EH):
                        e = h * EH + ei
                        bank = p2[ei // 2]
                        col = (ei % 2) * S
                        nc.tensor.matmul(
                            bank[:, col:col + S],
                            sb_ecN[b][:, e * P:(e + 1) * P], sb_tb[b][:],
                            start=(b == 0 and ei % 2 == 0),
                            stop=(b == NB - 1 and ei % 2 == 1))
                for ei in range(EH):
                    e = h * EH + ei
                    nc.vector.tensor_scalar_mul(
                        sb_oT[e][:], p2[ei // 2][:, (ei % 2) * S:(ei % 2) * S + S],
                        1.0 / B)
                if h == 0 and phases >= 2:
                    # m = mean(targets, 0) broadcast to all partitions, then
                    # pm[b] = (targets[b] - dg[b]) * m / B, overwriting dg.
                    # Emitted inside M2 so the tiny PE matmuls slot between
                    # quarters; pm is only needed by the M3 epilogues.
                    pmps = ps.tile([P, 512], F32, tag="mm", name="mmps")
                    for b in range(NB):
                        nc.tensor.matmul(
                            pmps[:1, :S], sb_ones[:], sb_tb[b][:],
                            start=(b == 0), stop=(b == NB - 1))
                    nc.vector.tensor_scalar_mul(
                        sb_mrow[:], pmps[:1, :S], 1.0 / B)
                    nc.gpsimd.partition_broadcast(sb_mbc[:], sb_mrow[:])
                    for b in range(NB):
                        nc.vector.tensor_sub(sb_dg[b][:], sb_tg[b][:],
                                             sb_dg[b][:])
                        nc.vector.tensor_mul(sb_dg[b][:], sb_dg[b][:],
                                             sb_mbc[:])

            # ---- M3: delta[b] = sum_k ecT[k,b]^T @ outerT[k] - pm[b], in
            # four waves of 2 groups so wave epilogues overlap the next
            # wave and only the last wave's two chains are exposed ----
            part_prev = None
            for w in range(4 if phases >= 4 else 0):
                p3 = [ps.tile([P, 512], F32, tag="mm", name="mmps")
                      for _ in range(2)]
                for k in range(KT):
                    for g in range(2):
                        nc.tensor.matmul(
                            p3[g][:, :S], ecT_sl(k, w * 2 + g), sb_oT[k][:],
                            start=(k == 0), stop=(k == KT - 1))
                for g in range(2):
                    b = w * 2 + g
                    d = tmp.tile([P, 512], F32, tag="d", name="d")
                    nc.vector.tensor_sub(d[:, :S], p3[g][:, :S], sb_dg[b][:])
                    dsq = tmp.tile([P, 512], F32, tag="dsq", name="dsq")
                    nc.vector.tensor_mul(dsq[:, :S], d[:, :S], d[:, :S])
                    part = tmp.tile([P, 1], F32, tag=f"part{b}",
                                    name=f"part{b}")
                    nc.vector.tensor_reduce(
                        part[:], dsq[:, :S], axis=mybir.AxisListType.X,
                        op=mybir.AluOpType.add)
                    if part_prev is not None:
                        nc.vector.tensor_add(part[:], part[:], part_prev[:])
                    part_prev = part

            # ---- write per-partition loss partials; host sums them ----
            if part_prev is not None:
                nc.sync.dma_start(loss[:, :], part_prev[:])
            else:
                zl = tmp.tile([P, 1], F32, tag="zl", name="zl")
                nc.vector.memset(zl[:], 0.0)
                nc.sync.dma_start(loss[:, :], zl[:])

    nc.compile()
    return nc


def _get_nc():
    if "nc" not in _CACHE:
        _CACHE["nc"] = _build()
    return _CACHE["nc"]


def _in_maps(ec, dg, W):
    ecN_h = ec.astype(MMNP)
    ecT_h = np.ascontiguousarray(ec.T).astype(MMNP)
    maps = []
    for i in range(N_CORES):
        Ws = W[i * S:(i + 1) * S, :]
        maps.append({
            "ecT": ecT_h,
            "ecN": ecN_h,
            "WT": np.ascontiguousarray(Ws.T).astype(MMNP),
            "dgN": np.ascontiguousarray(dg[:, i * S:(i + 1) * S]),
        })
    return maps


def kernel(ec_inputs: np.ndarray, dg_inputs: np.ndarray, W_ec: np.ndarray):
    nc = _get_nc()
    ec = np.asarray(ec_inputs, dtype=np.float32)
    dg = np.asarray(dg_inputs, dtype=np.float32)
    W = np.asarray(W_ec, dtype=np.float32)

    res = run_bass_kernel_spmd(nc, _in_maps(ec, dg, W), list(range(N_CORES)))

    pre_pc_cue = np.empty((B, CA3), dtype=np.float32)
    loss_sum = 0.0
    for i in range(N_CORES):
        pre_pc_cue[:, i * S:(i + 1) * S] = res.results[i]["outN"]
        loss_sum += float(res.results[i]["loss"].sum())

    loss = np.float32(LR * LR * loss_sum / (B * CA3))
    return (pre_pc_cue, np.float32(0.0), loss, loss)


# revision 31
# speedup vs baseline: 1.1285x; 1.0239x over previous
"""Trainium2 Bass kernel for the PerforantHebb AHA module.

Math (reference.py):
    pre     = ec @ W^T                      (B, CA3)
    targets = dg + pre                      -> output pre_pc_cue
    outer   = targets^T @ ec / B            (CA3, EC)
    m       = mean(targets, 0)              (CA3,)
    delta   = ec @ outer^T - pre * m[None]  (B, CA3)   [dW folded algebraically]
    ec_ca3_loss = pc_cue_loss = LR^2 * mean(delta^2);  dg_ca3_loss = 0

Sharding: CA3 split 8 ways (256 rows/core). Everything is local per core —
no collectives. Per core, three matmul phases, all with the batch dim on
PSUM partitions (no on-chip transposes):
  M1: pre[b]    = sum_k ecT[k,b]^T @ W_sT[k]      (K=EC)
  M2: outerT[e] = sum_b ecN[b,e]^T @ targets[b]   (K=B)
  M3: delta[b]  = sum_k ecT[k,b]^T @ outerT[k]    (K=EC)
Host feeds ec in both layouts (natural + transposed, bf16) plus per-core
W_sT / dg slices. M1 is k-outer with 8 concurrent psum groups so the PE
chases the ecT DMA stream; M2 runs in four e-quarters of 8 psum groups so
its first quarter chases the ecN stream.
"""

import numpy as np
import ml_dtypes

import concourse.bass as bass
import concourse.bacc as bacc
import concourse.bass_isa as bass_isa
import concourse.mybir as mybir
import concourse.tile as tile
from concourse.bass_utils import run_bass_kernel_spmd

B, EC, CA3 = 1024, 4096, 2048
LR = 0.01
N_CORES = 8
S = CA3 // N_CORES          # 256 ca3 rows per core
P = 128
KT = EC // P                # 32 k-tiles over EC
NB = B // P                 # 8 b-chunks of 128
KP = KT // 2                # k-pair DMA granules

F32 = mybir.dt.float32
MMDT = mybir.dt.bfloat16    # matmul operand dtype
MMNP = ml_dtypes.bfloat16

_CACHE = {}


def _build(phases=4):
    import os
    phases = int(os.environ.get("KERNEL_PHASES", phases))
    nc = bacc.Bacc("TRN2", target_bir_lowering=False, debug=False,
                   num_devices=N_CORES)
    ecT = nc.dram_tensor("ecT", (EC, B), MMDT, kind="ExternalInput").ap()
    ecN = nc.dram_tensor("ecN", (B, EC), MMDT, kind="ExternalInput").ap()
    WT = nc.dram_tensor("WT", (EC, S), MMDT, kind="ExternalInput").ap()
    dgN = nc.dram_tensor("dgN", (B, S), F32, kind="ExternalInput").ap()
    outN = nc.dram_tensor("outN", (B, S), F32, kind="ExternalOutput").ap()
    loss = nc.dram_tensor("loss", (P, 1), F32, kind="ExternalOutput").ap()

    with tile.TileContext(nc) as tc:
        with (
            tc.tile_pool(name="res", bufs=1) as res,
            tc.tile_pool(name="tmp", bufs=4) as tmp,
            tc.tile_pool(name="ps", bufs=8, space="PSUM") as ps,
        ):
            # ---- resident tiles ----
            sb_ecT = [res.tile([P, 2, B], MMDT, tag=f"ecT{kp}", name=f"ecT{kp}")
                      for kp in range(KP)]
            sb_WT = [res.tile([P, 2, S], MMDT, tag=f"WT{kp}", name=f"WT{kp}")
                     for kp in range(KP)]
            sb_ecN = [res.tile([P, EC], MMDT, tag=f"ecN{b}", name=f"ecN{b}")
                      for b in range(NB)]
            sb_dg = [res.tile([P, S], F32, tag=f"dg{b}", name=f"dg{b}")
                     for b in range(NB)]

            # dg early on the gpsimd (SWDGE) queue — parallel to the two
            # bandwidth-critical HWDGE queues (sync + scalar), which carry
            # ecT/WT pairs first (they pace M1) then ecN chunks (they pace
            # M2's first half). gpsimd DMAs all complete mid-kernel, so the
            # exit drain does not wait on them.
            qs = [nc.sync, nc.scalar]
            for b in range(NB):
                nc.gpsimd.dma_start(sb_dg[b][:], dgN[b * P:(b + 1) * P, :])
            for kp in range(KP):
                q = qs[kp % 2]
                q.dma_start(
                    sb_ecT[kp][:],
                    ecT[kp * 2 * P:(kp + 1) * 2 * P, :].rearrange(
                        "(two p) b -> p two b", p=P))
                q.dma_start(
                    sb_WT[kp][:],
                    WT[kp * 2 * P:(kp + 1) * 2 * P, :].rearrange(
                        "(two p) s -> p two s", p=P))
            for b in range(NB):
                qs[b % 2].dma_start(sb_ecN[b][:], ecN[b * P:(b + 1) * P, :])

            sb_tg = [res.tile([P, S], F32, tag=f"tg{b}", name=f"tg{b}")
                     for b in range(NB)]
            sb_tb = [res.tile([P, S], MMDT, tag=f"tb{b}", name=f"tb{b}")
                     for b in range(NB)]
            sb_oT = [res.tile([P, S], MMDT, tag=f"oT{k}", name=f"oT{k}")
                     for k in range(KT)]
            sb_ones = res.tile([P, 1], MMDT, tag="ones", name="ones")
            sb_mrow = res.tile([1, S], F32, tag="mrow", name="mrow")
            sb_mbc = res.tile([P, S], F32, tag="mbc", name="mbc")
            nc.vector.memset(sb_ones[:], 1.0)

            def ecT_sl(k, b):
                return sb_ecT[k // 2][:, k % 2, b * P:(b + 1) * P]

            # ---- M1: k-outer, 8 psum groups chase the ecT/WT stream ----
            p1 = [ps.tile([P, 512], F32, tag="mm", name="mmps")
                  for _ in range(NB)]
            for k in range(KT):
                for b in range(NB):
                    nc.tensor.matmul(
                        p1[b][:, :S], ecT_sl(k, b), sb_WT[k // 2][:, k % 2, :],
                        start=(k == 0), stop=(k == KT - 1))
            for b in range(NB):
                # targets = pre + dg (f32, main output) + bf16 copy for M2
                nc.vector.tensor_add(sb_tg[b][:], p1[b][:, :S], sb_dg[b][:])
                nc.vector.tensor_copy(sb_tb[b][:], sb_tg[b][:])
                nc.gpsimd.dma_start(outN[b * P:(b + 1) * P, :], sb_tg[b][:])

            # ---- M2: outerT[e] = sum_b ecN[b,e]^T @ targets[b], scaled 1/B.
            # Two e-halves of 16 concurrent psum groups: two (128,256)
            # groups share each bank. start=True zeroes the whole 2KiB
            # zero region, so only the bank's first MM sets start and only
            # its last sets stop; the odd group accumulates into the half
            # the even group's start already zeroed. 16 groups in flight
            # lets the first half consume the ecN stream at arrival rate.
            EH = 16
            for h in range(2 if phases >= 3 else 0):
                p2 = [ps.tile([P, 512], F32, tag="mm", name="mmps")
                      for _ in range(EH // 2)]
                for b in range(NB):
                    for ei in range(EH):
                        e = h * EH + ei
                        bank = p2[ei // 2]
                        col = (ei % 2) * S
                        nc.tensor.matmul(
                            bank[:, col:col + S],
                            sb_ecN[b][:, e * P:(e + 1) * P], sb_tb[b][:],
                            start=(b == 0 and ei % 2 == 0),
                            stop=(b == NB - 1 and ei % 2 == 1))
                for ei in range(EH):
                    e = h * EH + ei
                    nc.vector.tensor_scalar_mul(
                        sb_oT[e][:], p2[ei // 2][:, (ei % 2) * S:(ei % 2) * S + S],
                        1.0 / B)
                if h == 0 and phases >= 2:
                    # m = mean(targets, 0) broadcast to all partitions, then
                    # pm[b] = (targets[b] - dg[b]) * m / B, overwriting dg.
                    # Emitted inside M2 so the tiny PE matmuls slot between
                    # quarters; pm is only needed by the M3 epilogues.
                    pmps = ps.tile([P, 512], F32, tag="mm", name="mmps")
                    for b in range(NB):
                        nc.tensor.matmul(
                            pmps[:1, :S], sb_ones[:], sb_tb[b][:],
                            start=(b == 0), stop=(b == NB - 1))
                    nc.vector.tensor_scalar_mul(
                        sb_mrow[:], pmps[:1, :S], 1.0 / B)
                    nc.gpsimd.partition_broadcast(sb_mbc[:], sb_mrow[:])
                    for b in range(NB):
                        nc.vector.tensor_sub(sb_dg[b][:], sb_tg[b][:],
                                             sb_dg[b][:])
                        nc.vector.tensor_mul(sb_dg[b][:], sb_dg[b][:],
                                             sb_mbc[:])

            # ---- M3: delta[b] = sum_k ecT[k,b]^T @ outerT[k] - pm[b], in
            # four waves of 2 groups so wave epilogues overlap the next
            # wave and only the last wave's two chains are exposed ----
            part_prev = None
            for w in range(4 if phases >= 4 else 0):
                p3 = [ps.tile([P, 512], F32, tag="mm", name="mmps")
                      for _ in range(2)]
                for k in range(KT):
                    for g in range(2):
                        nc.tensor.matmul(
                            p3[g][:, :S], ecT_sl(k, w * 2 + g), sb_oT[k][:],
                            start=(k == 0), stop=(k == KT - 1))
                for g in range(2):
                    b = w * 2 + g
                    d = tmp.tile([P, 512], F32, tag="d", name="d")
                    nc.vector.tensor_sub(d[:, :S], p3[g][:, :S], sb_dg[b][:])
                    dsq = tmp.tile([P, 512], F32, tag="dsq", name="dsq")
                    nc.vector.tensor_mul(dsq[:, :S], d[:, :S], d[:, :S])
                    part = tmp.tile([P, 1], F32, tag=f"part{b}",
                                    name=f"part{b}")
                    nc.vector.tensor_reduce(
                        part[:], dsq[:, :S], axis=mybir.AxisListType.X,
                        op=mybir.AluOpType.add)
                    if part_prev is not None:
                        nc.vector.tensor_add(part[:], part[:], part_prev[:])
                    part_prev = part

            # ---- reduce loss partial across partitions on-chip and write a
            # single element (a (128,1) DMA is 128 scattered 4B descriptors
            # and costs ~6-7us of completion latency at kernel exit) ----
            if part_prev is None:
                part_prev = tmp.tile([P, 1], F32, tag="zl", name="zl")
                nc.vector.memset(part_prev[:], 0.0)
            lall = tmp.tile([P, 1], F32, tag="lall", name="lall")
            nc.gpsimd.partition_all_reduce(
                lall[:], part_prev[:], channels=P,
                reduce_op=bass_isa.ReduceOp.add)
            nc.sync.dma_start(loss[:1, :], lall[:1, :])

    nc.compile()
    return nc


def _get_nc():
    if "nc" not in _CACHE:
        _CACHE["nc"] = _build()
    return _CACHE["nc"]


def _in_maps(ec, dg, W):
    ecN_h = ec.astype(MMNP)
    ecT_h = np.ascontiguousarray(ec.T).astype(MMNP)
    maps = []
    for i in range(N_CORES):
        Ws = W[i * S:(i + 1) * S, :]
        maps.append({
            "ecT": ecT_h,
            "ecN": ecN_h,
            "WT": np.ascontiguousarray(Ws.T).astype(MMNP),
            "dgN": np.ascontiguousarray(dg[:, i * S:(i + 1) * S]),
        })
    return maps


def kernel(ec_inputs: np.ndarray, dg_inputs: np.ndarray, W_ec: np.ndarray):
    nc = _get_nc()
    ec = np.asarray(ec_inputs, dtype=np.float32)
    dg = np.asarray(dg_inputs, dtype=np.float32)
    W = np.asarray(W_ec, dtype=np.float32)

    res = run_bass_kernel_spmd(nc, _in_maps(ec, dg, W), list(range(N_CORES)))

    pre_pc_cue = np.empty((B, CA3), dtype=np.float32)
    loss_sum = 0.0
    for i in range(N_CORES):
        pre_pc_cue[:, i * S:(i + 1) * S] = res.results[i]["outN"]
        loss_sum += float(res.results[i]["loss"][0, 0])

    loss = np.float32(LR * LR * loss_sum / (B * CA3))
    return (pre_pc_cue, np.float32(0.0), loss, loss)
